# revision 25
# baseline (speedup 1.0000x reference)
"""Complex multihead attention (split softmax) on 8 Trainium2 NeuronCores.

Sharding: data-parallel over batch (B=4) x tensor-parallel over heads
(16 heads -> 2 groups of 8). core = b*2 + head_group. Each core computes
Q/K/V projections for its 8 heads, per-head attention, and a partial O
projection over its heads' columns; partials are summed on the host.

Device math notes (all validated against the reference in fp64/fp32):
 - K bias dropped: adds a per-query constant to logits -> softmax invariant.
 - V bias folded to a host-side constant: softmax rows sum to 1, so the
   bias contributes Wo @ ((1+i)*bv) to every output position.
 - Q bias applied during PSUM->SBUF evacuation (per-partition ACT bias).
 - All matmuls run as float32r (full-rate fp32 PE path).
 - Scores are computed transposed (St[sk, sq]) so softmax reduction over
   keys becomes a partition reduction done by ones-vector matmuls; the
   1/sum normalizer is broadcast across partitions with SBUF-SBUF DMAs.

Attention inner loop is software-pipelined one t-step: score matmuls for
step s issue before the sums/AV matmuls of step s-1, so the PE never
waits on the scalar-engine exp. PSUM: st x2 (2 banks), s_r/s_i packed in
one bank via partition-split matmuls (x2 gens = 2 banks), o1|o2 as one
[128,1024] pair tile (x2 gens = 4 banks).
"""

import ml_dtypes
import numpy as np

import concourse.bass as bass
from concourse import bacc
import concourse.mybir as mybir
import concourse.tile as tile
from concourse.bass_utils import run_bass_kernel_spmd

S, B, E, H, D = 1024, 4, 1024, 16, 64
HPC = 8            # heads per core
EH = HPC * D       # 512
N_CORES = 8
F32 = mybir.dt.float32
F32R = mybir.dt.float32r
BF16 = mybir.dt.bfloat16
AF = mybir.ActivationFunctionType

_NC_CACHE = []


def _r(ap):
    return ap


def _emit(tc):
    nc = tc.nc
    xq = nc.dram_tensor("xq", [3 * E, S], BF16, kind="ExternalInput").ap()
    xk = nc.dram_tensor("xk", [3 * E, S], BF16, kind="ExternalInput").ap()
    xv = nc.dram_tensor("xv", [3 * E, S], BF16, kind="ExternalInput").ap()
    wq = nc.dram_tensor("wq", [3 * E, EH], BF16, kind="ExternalInput").ap()
    wk = nc.dram_tensor("wk", [3 * E, EH], BF16, kind="ExternalInput").ap()
    wv = nc.dram_tensor("wv", [3 * E, EH], BF16, kind="ExternalInput").ap()
    wor = nc.dram_tensor("wor", [2 * EH, E], BF16, kind="ExternalInput").ap()
    woi = nc.dram_tensor("woi", [2 * EH, E], BF16, kind="ExternalInput").ap()
    bq = nc.dram_tensor("bq", [128, HPC], F32, kind="ExternalInput").ap()
    onesd = nc.dram_tensor("onesd", [128, 128], BF16, kind="ExternalInput").ap()
    ytr = nc.dram_tensor("ytr", [E, S], F32, kind="ExternalOutput").ap()
    yti = nc.dram_tensor("yti", [E, S], F32, kind="ExternalOutput").ap()

    store = tc.alloc_tile_pool(name="store", bufs=1)
    qcat = store.tile([128, HPC * S], BF16)   # per head j: [Qr;Qi]^T at cols j*S..
    kcat = store.tile([128, HPC * S], BF16)
    vnat = store.tile([128, HPC * S], BF16)   # per s-tile t: cols t*1024+(j,ri,d)
    bq_sb = store.tile([128, HPC], F32)
    ones_sq = store.tile([128, 128], BF16)
    nc.sync.dma_start(out=bq_sb, in_=bq)
    nc.sync.dma_start(out=ones_sq, in_=onesd)

    # attention-side SBUF pool; allocated early so head-prep DVE work can
    # overlap the V projection
    asb = tc.alloc_tile_pool(name="asb", bufs=2)
    vnat_v = vnat.rearrange("p (t j two d) -> p t j two d", t=8, j=HPC,
                            two=2, d=64)

    def prep_head_qv(j):
        """qv1 = [Qr; -Qi], qv2 = [Qi; Qr] for head j."""
        qh = qcat[:, j * S:(j + 1) * S]
        qv1 = asb.tile([128, S], BF16, tag="qv1", bufs=3, name=f"qv1_{j}")
        qv2 = asb.tile([128, S], BF16, tag="qv2", bufs=3, name=f"qv2_{j}")
        nc.vector.tensor_copy(qv1[0:64, :], qh[0:64, :])
        nc.vector.tensor_scalar_mul(qv1[64:128, :], qh[64:128, :], -1.0)
        # partition swap -> SBUF-to-SBUF DMA
        nc.sync.dma_start(out=qv2[0:64, :], in_=qh[64:128, :])
        nc.sync.dma_start(out=qv2[64:128, :], in_=qh[0:64, :])
        return qv1, qv2

    def prep_head_vsw(j):
        """per-head [Vi|Vr] swapped copy for the o2 products."""
        vsw = asb.tile([128, 8, 2, 64], BF16, tag="vsw", bufs=3,
                       name=f"vsw_{j}")
        nc.vector.tensor_copy(vsw[:, :, 0, :], vnat_v[:, :, j, 1, :])
        nc.vector.tensor_copy(vsw[:, :, 1, :], vnat_v[:, :, j, 0, :])
        return vsw

    # ---------------- projections ----------------
    with tc.tile_pool(name="xp", bufs=24) as xp, \
         tc.tile_pool(name="wp", bufs=24) as wp, \
         tc.tile_pool(name="pp", bufs=8, space="PSUM") as pp:

        # Q/K projections via Karatsuba. Host ships x = [Xr; Xi; Xr+Xi] and
        # w = [Wr; Wi; Wr+Wi]; products P1 = Xr Wr, P2 = Xi Wi,
        # P3 = (Xr+Xi)(Wr+Wi) give re = P1-P2, im = P3-P1-P2. Product rows
        # hold head pairs (2m, 2m+1); the rows whose destination is in the
        # other partition half descend via SBUF-to-SBUF DMA.
        SUB = mybir.AluOpType.subtract
        ADD = mybir.AluOpType.add
        for which, xdram, wdram, dest, isq in (
            ("q", xq, wq, qcat, True),
            ("k", xk, wk, kcat, False),
        ):
            xs = []
            wts = []
            for pk in range(24):
                wt = wp.tile([128, 512], BF16, tag="w", name=f"w{which}{pk}")
                nc.sync.dma_start(out=wt,
                                  in_=wdram[pk * 128:(pk + 1) * 128, :])
                wts.append(wt)
                xt = xp.tile([128, S], BF16, tag="x", name=f"x{which}{pk}")
                nc.sync.dma_start(out=xt, in_=xdram[pk * 128:(pk + 1) * 128, :])
                xs.append(xt)
            for m in range(4):
                for sh in range(2):
                    ps = [pp.tile([128, 512], F32, tag="pp",
                                  name=f"p{which}{m}{sh}{p}") for p in range(3)]
                    for p in range(3):
                        for k in range(8):
                            nc.tensor.matmul(
                                ps[p],
                                _r(wts[p * 8 + k][:, m * 128:(m + 1) * 128]),
                                _r(xs[p * 8 + k][:, sh * 512:(sh + 1) * 512]),
                                start=(k == 0), stop=(k == 7))
                    p1, p2, p3 = ps
                    # stage t1 = P1 (+ b_r for Q); t3 = P3 (+ b_r+b_i) - t1
                    t1 = asb.tile([128, 512], F32, tag="kt1", name=f"t1{which}{m}{sh}")
                    t3 = asb.tile([128, 512], F32, tag="kt3", name=f"t3{which}{m}{sh}")
                    if isq:
                        nc.vector.tensor_scalar(t1, p1, bq_sb[:, m:m + 1],
                                                None, op0=ADD)
                        nc.vector.scalar_tensor_tensor(
                            t3, p3, bq_sb[:, 4 + m:5 + m], t1,
                            op0=ADD, op1=SUB)
                    else:
                        nc.vector.tensor_copy(t1, p1)
                        nc.vector.tensor_sub(t3, p3, t1)
                    je, jo = 2 * m, 2 * m + 1
                    ce = slice(je * S + sh * 512, je * S + (sh + 1) * 512)
                    co = slice(jo * S + sh * 512, jo * S + (sh + 1) * 512)
                    # direct writes: re(even) at rows 0:64, im(odd) at 64:128
                    nc.vector.tensor_sub(dest[0:64, ce], t1[0:64, :],
                                         p2[0:64, :])
                    nc.vector.tensor_sub(dest[64:128, co], t3[64:128, :],
                                         p2[64:128, :])
                    # crossing writes: im(even) / re(odd) via DMA descent
                    tmp = asb.tile([128, 512], BF16, tag="ktx",
                                   name=f"tx{which}{m}{sh}")
                    nc.vector.tensor_sub(tmp[0:64, :], t3[0:64, :],
                                         p2[0:64, :])
                    nc.vector.tensor_sub(tmp[64:128, :], t1[64:128, :],
                                         p2[64:128, :])
                    nc.sync.dma_start(out=dest[64:128, ce], in_=tmp[0:64, :])
                    nc.sync.dma_start(out=dest[0:64, co], in_=tmp[64:128, :])

        # head 0/1 qv prep: DVE work that overlaps the V matmuls below
        qv01 = [prep_head_qv(0), prep_head_qv(1)]

        # V projection via Karatsuba: P1 = Xr Wr, P2 = Xi Wi,
        # P3 = (Xr+Xi)(Wr+Wi); Vr = P1-P2, Vi = P3-P1-P2. The host ships
        # xv = [Xr; Xi; Xr+Xi] and wv = [Wr; Wi; Wr+Wi] stacked.
        xs = []
        wts = []
        for pk in range(24):
            wt = wp.tile([128, 512], BF16, tag="w", name=f"wv{pk}")
            nc.sync.dma_start(out=wt, in_=wv[pk * 128:(pk + 1) * 128, :])
            wts.append(wt)
            xt = xp.tile([128, S], BF16, tag="x", name=f"xv{pk}")
            nc.sync.dma_start(out=xt, in_=xv[pk * 128:(pk + 1) * 128, :])
            xs.append(xt)
        for st in range(8):
            ps = [pp.tile([128, 512], F32, tag="pp", name=f"pv{p}{st}")
                  for p in range(3)]
            for p in range(3):
                for k in range(8):
                    nc.tensor.matmul(
                        ps[p], _r(xs[p * 8 + k][:, st * 128:(st + 1) * 128]),
                        _r(wts[p * 8 + k][:, :]),
                        start=(k == 0), stop=(k == 7))
            p1, p2, p3 = ps
            # DVE may read only one PSUM operand per op: stage P1 in SBUF
            t1 = asb.tile([128, 512], F32, tag="vt1", name=f"vt1{st}")
            nc.vector.tensor_copy(t1, p1)
            nc.vector.tensor_sub(vnat_v[:, st, :, 0, :], t1, p2)
            t3 = asb.tile([128, 512], F32, tag="vt3", name=f"vt3{st}")
            nc.vector.tensor_sub(t3, p3, t1)
            nc.vector.tensor_sub(vnat_v[:, st, :, 1, :], t3, p2)

    # ---------------- attention ----------------
    attn_pool = tc.alloc_tile_pool(name="attnp", bufs=1)
    attn = attn_pool.tile([128, HPC * S], BF16)  # per head j: [or;oi]^T

    # O-projection weights prefetch pool (consumed in the next phase); the
    # gathered DMAs have no upstream deps so they fill DMA idle time here.
    wop = tc.alloc_tile_pool(name="wop", bufs=4)
    wo_tiles = {}
    for part, wo_d in ((0, wor), (1, woi)):
        wo_re = wo_d.rearrange("(j p) n -> p j n", p=128)  # [128, 8, 1024]
        for m in range(8):
            wt = wop.tile([128, 8, 128], BF16, tag="wo", name=f"wo{part}{m}")
            nc.sync.dma_start(out=wt, in_=wo_re[:, :, m * 128:(m + 1) * 128])
            wo_tiles[(part, m)] = wt

    with tc.tile_pool(name="stp", bufs=2, space="PSUM") as stp, \
         tc.tile_pool(name="opp", bufs=2, space="PSUM") as opp, \
         tc.tile_pool(name="smp", bufs=1, space="PSUM") as smp:

        def finalize(fin):
            """Normalize + combine for a finished (j, qi) iteration."""
            j, qi, s, op = fin
            sq0 = qi * 512
            # s holds [sum_r | sum_i], already replicated on all partitions
            rcp = asb.tile([128, 1024], F32, tag="rcp", name=f"rc{j}{qi}")
            nc.vector.reciprocal_approx_fast(rcp, s)
            t12 = asb.tile([128, 1024], F32, tag="t12", name=f"t12{j}{qi}")
            nc.vector.tensor_mul(t12, op, rcp)
            dst = attn[:, j * S + sq0: j * S + sq0 + 512]
            # real rows: (PrVr)^T/sum_r - (PiVi)^T/sum_i
            nc.vector.tensor_sub(dst[0:64, :], t12[0:64, 0:512],
                                 t12[0:64, 512:1024])
            # imag rows: (PrVi)^T/sum_r + (PiVr)^T/sum_i
            nc.vector.tensor_add(dst[64:128, :], t12[64:128, 0:512],
                                 t12[64:128, 512:1024])

        def prev_work(p):
            """Sums + AV matmuls for a step whose exp is already in flight.
            The ones lhsT is M=128 wide, so the sums land replicated on all
            128 partitions -- no cross-partition broadcast needed later."""
            j, qi, t, pt_r, pt_i, s, op, vsw = p
            nc.tensor.matmul(s[:, 0:512], ones_sq, pt_r,
                             start=(t == 0), stop=(t == 7))
            nc.tensor.matmul(s[:, 512:1024], ones_sq, pt_i,
                             start=(t == 0), stop=(t == 7),
                             skip_group_check=True)
            vl = vnat[:, t * 1024 + j * 128: t * 1024 + (j + 1) * 128]
            nc.tensor.matmul(op[:, 0:512], vl, pt_r,
                             start=(t == 0), stop=(t == 7),
                             skip_group_check=True)
            nc.tensor.matmul(op[:, 512:1024],
                             vsw.rearrange("p t two d -> p (t two d)")
                             [:, t * 128:(t + 1) * 128], pt_i,
                             start=(t == 0), stop=(t == 7),
                             skip_group_check=True)

        steps = [(j, qi, t) for j in range(HPC) for qi in range(2)
                 for t in range(8)]
        head_ctx = {0: qv01[0] + (prep_head_vsw(0),),
                    1: qv01[1] + (prep_head_vsw(1),)}
        prev = None          # (j, qi, t, pt_r, pt_i, s, op, vsw)
        s = op = None
        for j, qi, t in steps:
            if qi == 1 and t == 0 and j + 2 < HPC:
                head_ctx[j + 2] = prep_head_qv(j + 2) + (prep_head_vsw(j + 2),)
            qv1, qv2, vsw = head_ctx[j]
            sq0 = qi * 512
            if t == 0:
                s = smp.tile([128, 1024], F32, tag="sums", name=f"s_{j}{qi}")
                op = opp.tile([128, 1024], F32, tag="opv", name=f"o_{j}{qi}")
            # score matmuls for this step (issue ahead of prev step's work)
            st_r = stp.tile([128, 512], F32, tag="st", name=f"str{j}{qi}{t}")
            st_i = stp.tile([128, 512], F32, tag="st", name=f"sti{j}{qi}{t}")
            kl = kcat[:, j * S + t * 128: j * S + (t + 1) * 128]
            nc.tensor.matmul(st_r, kl, qv1[:, sq0:sq0 + 512],
                             start=True, stop=True)
            nc.tensor.matmul(st_i, kl, qv2[:, sq0:sq0 + 512],
                             start=True, stop=True)
            pt_r = asb.tile([128, 512], BF16, tag="pt", bufs=6,
                            name=f"ptr{j}{qi}{t}")
            pt_i = asb.tile([128, 512], BF16, tag="pt", bufs=6,
                            name=f"pti{j}{qi}{t}")
            nc.scalar.activation(pt_r, st_r, AF.Exp, scale=0.125)
            nc.scalar.activation(pt_i, st_i, AF.Exp, scale=0.125)
            # sums + AV matmuls for the PREVIOUS step (pt already in flight)
            if prev is not None:
                prev_work(prev)
                if prev[2] == 7:
                    finalize((prev[0], prev[1], prev[5], prev[6]))
            prev = (j, qi, t, pt_r, pt_i, s, op, vsw)
        prev_work(prev)
        finalize((prev[0], prev[1], prev[5], prev[6]))

    # ---------------- O projection (partials) ----------------
    with tc.tile_pool(name="ytp", bufs=4) as ytp, \
         tc.tile_pool(name="pop", bufs=4, space="PSUM") as pop:
        for part, wo_d, yt_d in ((0, wor, ytr), (1, woi, yti)):
            for m in range(8):
                wt = wo_tiles[(part, m)]
                for hf in range(2):
                    ps = pop.tile([128, 512], F32, tag="po", name=f"po{part}{m}{hf}")
                    for jj in range(8):
                        nc.tensor.matmul(
                            ps, _r(wt[:, jj, :]),
                            _r(attn[:, jj * S + hf * 512: jj * S + (hf + 1) * 512]),
                            start=(jj == 0), stop=(jj == 7))
                    yt_t = ytp.tile([128, 512], F32, tag="yt", name=f"yt{part}{m}{hf}")
                    nc.vector.tensor_copy(yt_t, ps)
                    nc.sync.dma_start(
                        out=yt_d[m * 128:(m + 1) * 128, hf * 512:(hf + 1) * 512],
                        in_=yt_t)

    wop.release()
    attn_pool.release()
    asb.release()
    store.release()


def build_module():
    nc = bacc.Bacc("TRN2", target_bir_lowering=False)
    with tile.TileContext(nc) as tc:
        _emit(tc)
    nc.compile()
    return nc


def _get_nc():
    if not _NC_CACHE:
        _NC_CACHE.append(build_module())
    return _NC_CACHE[0]


def prep_core(inp, core):
    """Host-side shard prep for one core."""
    b, hg = divmod(core, 2)
    hs, he = hg * EH, (hg + 1) * EH

    def wo_prep(w_top, w_bot):
        Ct = w_top[:, hs:he].T.reshape(HPC, D, E)
        Dt = w_bot[:, hs:he].T.reshape(HPC, D, E)
        return np.ascontiguousarray(
            np.concatenate([Ct, Dt], axis=1).reshape(2 * EH, E),
            dtype=ml_dtypes.bfloat16)

    # bias pack: cols 0..3 = b_r for head pairs (2m, 2m+1); cols 4..7 = b_r+b_i
    bqp = np.empty((128, HPC), np.float32)
    bri = inp["bq_r"] + inp["bq_i"]
    for m in range(4):
        he_, ho_ = hg * HPC + 2 * m, hg * HPC + 2 * m + 1
        bqp[:64, m] = inp["bq_r"][he_ * D:(he_ + 1) * D]
        bqp[64:, m] = inp["bq_r"][ho_ * D:(ho_ + 1) * D]
        bqp[:64, 4 + m] = bri[he_ * D:(he_ + 1) * D]
        bqp[64:, 4 + m] = bri[ho_ * D:(ho_ + 1) * D]

    def xcat3(xr, xi):
        a = xr[:, b, :].T
        c = xi[:, b, :].T
        return np.ascontiguousarray(
            np.concatenate([a, c, a + c], axis=0), dtype=ml_dtypes.bfloat16)

    def wv3_prep(wr, wi):
        W1 = wr[hs:he, :].T
        W2 = wi[hs:he, :].T
        return np.ascontiguousarray(
            np.concatenate([W1, W2, W1 + W2], axis=0), dtype=ml_dtypes.bfloat16)

    return dict(
        xq=xcat3(inp["query_r"], inp["query_i"]),
        xk=xcat3(inp["key_r"], inp["key_i"]),
        xv=xcat3(inp["value_r"], inp["value_i"]),
        wq=wv3_prep(inp["wq_r"], inp["wq_i"]),
        wk=wv3_prep(inp["wk_r"], inp["wk_i"]),
        wv=wv3_prep(inp["wv_r"], inp["wv_i"]),
        wor=wo_prep(inp["wo_r"], -inp["wo_i"]),
        woi=wo_prep(inp["wo_i"], inp["wo_r"]),
        bq=bqp,
        onesd=np.ones((128, 128), ml_dtypes.bfloat16),
    )


def host_combine(results, inp):
    """Sum per-core partials, add the host-side constant, untranspose."""
    bvr = inp["bv_r"].astype(np.float64)
    bvi = inp["bv_i"].astype(np.float64)
    wr = inp["wo_r"].astype(np.float64)
    wi = inp["wo_i"].astype(np.float64)
    vb_r = bvr - bvi
    vb_i = bvr + bvi
    yc_r = (wr @ vb_r - wi @ vb_i + inp["bo_r"]).astype(np.float32)
    yc_i = (wr @ vb_i + wi @ vb_r + inp["bo_i"]).astype(np.float32)

    out = np.empty((S, B, E, 2), np.float32)
    for b in range(B):
        yr = results[2 * b]["ytr"] + results[2 * b + 1]["ytr"]
        yi = results[2 * b]["yti"] + results[2 * b + 1]["yti"]
        out[:, b, :, 0] = yr.T + yc_r
        out[:, b, :, 1] = yi.T + yc_i
    return out


def kernel(**inputs):
    inputs = {k: np.asarray(v) for k, v in inputs.items()}
    nc = _get_nc()
    in_maps = [prep_core(inputs, c) for c in range(N_CORES)]
    res = run_bass_kernel_spmd(nc, in_maps, core_ids=list(range(N_CORES)))
    return host_combine(res.results, inputs)
